# revision 31
# baseline (speedup 1.0000x reference)
"""Trainium2 Bass kernel for nn_Interpolator: pilot-to-subcarrier linear
interpolation with learned per-subcarrier weights.

Math: out[b, t] = alpha[t] * Hp[b, right[t]] + beta[t] * Hp[b, left[t]]
where Hp = [H, extrapolated last column]. The op is linear in H, so it
collapses to out = H @ W with a sparse W [256, 4096] built on the host
from (pilot_loc, alpha, beta); the extrapolation column folds into W's
last two rows.

Compression: output column t only depends on column t of W, so columns
of W that are identical produce identical output columns. The device
computes out_u = H @ Wu for the U *unique* columns of W only, and the
host scatters out_u's columns back to the full 4096 (pure indexing).
For this module's inputs (constant alpha/beta, stride-16 pilots) U=256,
a 16x cut in device output traffic. Falls back gracefully (same code
path) to any U up to 4096.

Precision: H and Wu are sent in bf16; out_u is stored in bf16. Error is
~2e-3 relative (bf16 rounding in, bf16 rounding out) against a 2e-2
gate. If Wu is not exactly bf16-representable a Wu_lo correction term
is added (for this module's inputs Wu is exact in bf16, so it is not).

Compute: per (batch tile, component, W chunk) the K=256 contraction
runs as two accumulating K=128 matmuls (one per 128-row half of Wu)
into one PSUM bank. (DoubleRow would fuse the halves but is fp8-only,
and fp8 needs a hi+lo pair for the 2e-2 gate - a wash.)

Schedule (from NTFF traces: ~135GB/s per DMA queue at 1-4KB rows,
queue cold-start 1.5-3.3us, trigger->data ~1.4us):
- H^T is pre-transposed on the host, packed tile-major ([x0h0 x0h1
  x1h0 x1h1] 512-col blocks), W prepended so one early sync-ring DMA
  delivers [W | t0 | t1]; remaining input spans spread over the three
  rings (sync/scalar/gpsimd), a dummy read pre-warms gpsimd's queue.
- One fp32->bf16 cast per PSUM tile, alternating DVE/ACT; output
  drains in 2-tile batches round-robining all three rings with
  single-tile stores at the tail.

Sharding: data-parallel over the batch dim, 2048 rows per core x 8.
"""

import os
import sys

if os.path.isdir("/opt/trn_rl_repo") and "/opt/trn_rl_repo" not in sys.path:
    sys.path.insert(0, "/opt/trn_rl_repo")

import ml_dtypes
import numpy as np

_BF16 = np.dtype(ml_dtypes.bfloat16)

_B, _P, _NFFT = 16384, 256, 4096
_NC = 8
_BS = _B // _NC          # rows per core
_PT = 128                # partition tile (batch rows per tile)
_NBT = _BS // _PT        # batch tiles per core
_CH = 512                # max PSUM chunk width (one bank of fp32)

# Input-load groups (start_tile, n_tiles, ring): ring 0=sync,
# 1=gpsimd, 2=scalar. Group 0's DMA also carries W (prepended in the
# same DRAM tensor) so one early sync DMA delivers everything the
# first two tiles need; gpsimd (slowest wake, ~3.3us) gets a mid group
# after a warm-up dummy; the last tiles ride scalar once its first
# span drains.
_IN_GROUPS = [(0, 2, 0), (2, 4, 2), (6, 4, 0), (10, 4, 1), (14, 2, 2)]
# Output-store batches (start_tile, n_tiles, ring): 2-tile batches
# round-robin the rings in the order they come free from input loads,
# with single-tile stores at the tail so the drain ends with the last
# cast.
_OUT_BATCHES = [(0, 2, 1), (2, 2, 0), (4, 2, 2), (6, 2, 1), (8, 2, 0),
                (10, 2, 2), (12, 1, 1), (13, 1, 0), (14, 1, 2),
                (15, 1, 1)]

_cache = {}


def _interp_matrix(pilot_loc, alpha, beta):
    """W [256, 4096] f32 such that out = H @ W reproduces the reference."""
    p = pilot_loc.astype(np.float64) - 1.0  # reference: 1-based -> 0-based
    pp = np.concatenate([p, [float(_NFFT - 1)]])
    t = np.arange(_NFFT)
    left = np.clip(np.searchsorted(pp, t, side="right") - 1, 0, _P - 1)
    right = left + 1
    Wf = np.zeros((_P + 1, _NFFT), np.float64)
    Wf[left, t] += beta.astype(np.float64)
    Wf[right, t] += alpha.astype(np.float64)
    # Hp[:, P] = H[:, P-1] + slope * (NFFT-1 - p[-1]),
    # slope = (H[:, P-1] - H[:, P-2]) / (p[-1] - p[-2])  -> linear in H.
    d = (float(_NFFT - 1) - p[-1]) / (p[-1] - p[-2])
    W = Wf[:_P]
    W[_P - 1] += (1.0 + d) * Wf[_P]
    W[_P - 2] += (-d) * Wf[_P]
    return np.ascontiguousarray(W.astype(np.float32))


def _unique_cols(W):
    """Wu [256, U] = unique columns of W; inv [4096] with W = Wu[:, inv]."""
    uniq, inv = np.unique(W.T, axis=0, return_inverse=True)
    return np.ascontiguousarray(uniq.T), inv.astype(np.int64).ravel()


def _chunks(U):
    """<=512-col chunks of Wu's column space."""
    return tuple((c * _CH, min(U, (c + 1) * _CH))
                 for c in range((U + _CH - 1) // _CH))


def _bf16_split(x):
    hi = x.astype(_BF16)
    lo = (x - hi.astype(np.float32)).astype(_BF16)
    return hi, lo


def _build_program(U, use_wlo):
    from contextlib import ExitStack

    import concourse.bacc as bacc
    import concourse.mybir as mybir
    import concourse.tile as tile

    f32 = mybir.dt.float32
    bf16 = mybir.dt.bfloat16
    chunks = _chunks(U)

    nc = bacc.Bacc("TRN2", target_bir_lowering=False, debug=False,
                   num_devices=_NC)
    # One packed DRAM input: [ W | Wlo (if used) | tile-major H^T ].
    # W is chunk-major with halves adjacent: for chunk (lo, hi),
    # cols [2*lo + 0 : +cw] = Wu[half0, lo:hi], [2*lo + cw :] = half1
    # (the DoubleRow moving operand wants (half, col) pairs adjacent).
    # H^T tile block bt: [x0h0 | x0h1 | x1h0 | x1h1] x 128 batch cols.
    n_w = 2 if use_wlo else 1
    w_cols = n_w * 2 * U
    h_in = nc.dram_tensor("hxt", [128, w_cols + 4 * _BS], bf16,
                          kind="ExternalInput").ap()
    # Tile-major output: batch tile bt at cols [bt*2U, (bt+1)*2U).
    out = nc.dram_tensor("out", [128, _NBT * 2 * U], bf16,
                         kind="ExternalOutput").ap()

    # Pack (comp, chunk) groups into <=512-wide PSUM tiles.
    groups = [(x, lo, hi) for x in (0, 1) for (lo, hi) in chunks]
    ps_specs, cur, curw = [], [], 0
    for g in groups:
        w = g[2] - g[1]
        if curw + w > _CH and cur:
            ps_specs.append((tuple(cur), curw))
            cur, curw = [], 0
        cur.append(g)
        curw += w
    ps_specs.append((tuple(cur), curw))

    tile_group = {}  # bt -> group_idx
    for gi, (s, n, _r) in enumerate(_IN_GROUPS):
        for j in range(n):
            tile_group[s + j] = gi

    with tile.TileContext(nc) as tc, ExitStack() as ctx:
        const_pool = ctx.enter_context(tc.tile_pool(name="const", bufs=1))
        out_pool = ctx.enter_context(
            tc.tile_pool(name="outp", bufs=len(_OUT_BATCHES)))
        ps_mm = ctx.enter_context(tc.tile_pool(name="psm", bufs=8,
                                               space="PSUM"))

        rings = [nc.sync, nc.gpsimd, nc.scalar]

        # Group 0's DMA carries [W | t0 | t1] in one transfer on sync
        # (fastest queue wake); a tiny dummy read warms the gpsimd
        # queue so its first real DMA skips the ~3.3us cold start.
        hx = [None] * len(_IN_GROUPS)   # (tile, base col of group)
        w_sb = {}

        s0, n0, r0 = _IN_GROUPS[0]
        t0 = const_pool.tile([128, w_cols + n0 * 512], bf16, tag="wg0")
        rings[r0].dma_start(t0[:], h_in[:, :w_cols + n0 * 512])
        w_sb["h"] = (t0, 0)
        if use_wlo:
            w_sb["l"] = (t0, 2 * U)
        hx[0] = (t0, w_cols)

        warm = const_pool.tile([128, 4], bf16, tag="warm")
        nc.gpsimd.dma_start(warm[:], h_in[:, 0:4])

        for gi in range(1, len(_IN_GROUPS)):
            s, n, r = _IN_GROUPS[gi]
            t = const_pool.tile([128, n * 512], bf16, tag=f"hx{gi}")
            rings[r].dma_start(t[:], h_in[:, w_cols + s * 512:
                                          w_cols + (s + n) * 512])
            hx[gi] = (t, -s * 512)  # so base + bt*512 indexes the tile

        terms = ["h"] if not use_wlo else ["h", "l"]
        cast_idx = 0
        for (s, n, ring) in _OUT_BATCHES:
            ot = out_pool.tile([128, n * 2 * U], bf16, tag=f"ot{n}")
            for j2 in range(n):
                bt = s + j2
                ht, hb = hx[tile_group[bt]]
                off = j2 * 2 * U
                for (grs, wdt) in ps_specs:
                    ps = ps_mm.tile([128, wdt], f32, tag="ps")
                    poff = 0
                    for (x, clo, chi) in grs:
                        cw = chi - clo
                        lo_ = hb + 512 * bt + 256 * x
                        n_mm = 2 * len(terms)
                        kk = 0
                        for h in (0, 1):
                            for wp in terms:
                                wt, wb = w_sb[wp]
                                nc.tensor.matmul(
                                    ps[:, poff:poff + cw],
                                    ht[:, lo_ + 128 * h:
                                       lo_ + 128 * h + 128],
                                    wt[:, wb + 2 * clo + h * cw:
                                       wb + 2 * clo + (h + 1) * cw],
                                    start=(kk == 0),
                                    stop=(kk == n_mm - 1),
                                )
                                kk += 1
                        poff += cw
                    dst = ot[:, off:off + wdt]
                    if cast_idx % 2 == 0:
                        nc.vector.tensor_copy(dst, ps[:])
                    else:
                        nc.scalar.copy(dst, ps[:])
                    cast_idx += 1
                    off += wdt
            rings[ring].dma_start(
                out[:, s * 2 * U:(s + n) * 2 * U], ot[:])

    nc.compile()
    return nc


def _get_program(U, use_wlo):
    key = (U, use_wlo)
    prog = _cache.get(key)
    if prog is None:
        prog = _build_program(U, use_wlo)
        _cache[key] = prog
    return prog


def _prepare(H_real, H_imag, pilot_loc, alpha, beta):
    """Build (nc, in_maps, (U, inv)) for the spmd run."""
    H_real = np.ascontiguousarray(np.asarray(H_real, dtype=np.float32))
    H_imag = np.ascontiguousarray(np.asarray(H_imag, dtype=np.float32))
    pilot_loc = np.asarray(pilot_loc, dtype=np.float32)
    alpha = np.asarray(alpha, dtype=np.float32)
    beta = np.asarray(beta, dtype=np.float32)

    W = _interp_matrix(pilot_loc, alpha, beta)
    Wu, inv = _unique_cols(W)
    U = Wu.shape[1]
    wu_hi, wu_lo = _bf16_split(Wu)
    use_wlo = bool(np.any(np.asarray(wu_lo) != 0))
    nc = _get_program(U, use_wlo)

    def pack_w(w):
        # chunk-major, halves adjacent within each chunk
        w = np.asarray(w).reshape(2, 128, U)    # [half, part, col]
        blocks = []
        for (lo, hi) in _chunks(U):
            blocks.append(w[0, :, lo:hi])
            blocks.append(w[1, :, lo:hi])
        return np.concatenate(blocks, axis=1)   # [128, 2U]

    # Transposed bf16 inputs -> per-core packing [W | (Wlo) | tiles]:
    # tile block bt = [x0h0 | x0h1 | x1h0 | x1h1] x 128 batch cols.
    X = np.stack([H_real.astype(_BF16).T, H_imag.astype(_BF16).T])
    X = np.ascontiguousarray(X)                 # [2, 256, B]

    wblocks = [pack_w(wu_hi)]
    if use_wlo:
        wblocks.append(pack_w(wu_lo))

    in_maps = []
    for i in range(_NC):
        slab = X[:, :, i * _BS:(i + 1) * _BS]   # [2, 256, BS]
        ht = (slab.reshape(2, 2, 128, _NBT, 128)  # x h part bt col
              .transpose(2, 3, 0, 1, 4)           # part bt x h col
              .reshape(128, 4 * _BS))
        hxt = np.concatenate(wblocks + [ht], axis=1)
        in_maps.append({"hxt": np.ascontiguousarray(hxt)})
    return nc, in_maps, (U, inv)


def _assemble(results, U, inv):
    full = np.concatenate(
        [np.asarray(r["out"]).reshape(128, _NBT, 2 * U).transpose(1, 0, 2)
         .reshape(_BS, 2 * U) for r in results], axis=0)
    full = full.astype(np.float32)          # [B, 2U]: [real | imag]
    idx = np.empty(2 * _NFFT, np.int64)     # interleave (r, i) per t
    idx[0::2] = inv
    idx[1::2] = U + inv
    return full[:, idx].reshape(_B, _NFFT, 2)


def kernel(H_real, H_imag, pilot_loc, alpha, beta):
    nc, in_maps, (U, inv) = _prepare(H_real, H_imag, pilot_loc,
                                     alpha, beta)
    from concourse.bass_utils import run_bass_kernel_spmd

    res = run_bass_kernel_spmd(nc, in_maps, list(range(_NC))).results
    return _assemble(res, U, inv)
